# revision 8
# baseline (speedup 1.0000x reference)
"""Trainium2 Bass kernel for nn_CNFBlock — v2, instruction-count-minimized.

Contract: kernel(**inputs) takes FULL unsharded inputs (numpy), returns the
FULL output [16, 10000] float32.

Reformulation (exact algebra vs the reference RK4):
  * State tracked in pre-activation space q = Wx@z~ + hb, where z~ excludes
    the b2/time drift (z = z~ + t*b2); the drift enters only through the
    per-eval ACT bias t_i*v with v = wxt + wht + Wx@b2.
  * One weight matrix M = Wx@W2 serves the whole RK step:
      q_{i+1} = base + a_i * M@sp_i   (a in {dt/2, dt/2, dt})
      base'   = base + (dt/6) * M@(sp1 + 2sp2 + 2sp3 + sp4)
    i.e. 4 matmul passes per step instead of 8 (scaled stationaries
    M_half, M_dt, M_16 are pre-scaled on host; S is combined on DVE).
  * Per eval, one table set (natural_log_exp_and_others): e = Exp(q + t*v),
    sp = Ln(e + 1) = softplus, s2 = Exp(-sp) = 1 - sigmoid.
  * Divergence: acc += s2 *pp (w_i * (dt/6) * c) (per-partition scalar in
    the stt), then ONE gpsimd partition_all_reduce over E gives the token
    row; host computes out = log_pz0 - sum(c) + sum_p acc.
  * Sharding: core c handles tokens [1250c, 1250(c+1)) for ALL 16 sb rows
    (width 16*1250 = 20000 per core), processed in 4 stripes of 5000
    (4 sb each). PSUM macro width 2500 (5 banks).
  * repeat>1 is a tc.For_i hardware loop: the body is emitted once and
    re-executed on device, so per-iteration cost is true dynamic execution
    (the gpsimd library + act-table loads are hoisted out of the loop).
"""

import sys

for _p in ("/opt/trn_rl_repo", "/root/.axon_site/_ro/trn_rl_repo"):
    if _p not in sys.path:
        sys.path.append(_p)

import numpy as np

import concourse.bacc as bacc
import concourse.tile as tile
from concourse import mybir
from concourse import bass_isa
from concourse.bass_utils import run_bass_kernel_spmd

AF = mybir.ActivationFunctionType

N_CORES = 8
SB = 16
T = 10000
E = 128
DT = 0.5                  # T_END / N_STEPS
TOK = T // N_CORES        # 1250 tokens per core
W_STRIPE = 4 * TOK        # 5000 (4 sb rows)
N_STRIPES = 4
W_MACRO = 2500            # PSUM macro (5 banks)

_F32 = mybir.dt.float32
_F32R = mybir.dt.float32r

# Activation-table patch (same trick as the baseline kernel): this kernel
# only uses Exp / Ln / Identity, which all live in one table set
# (natural_log_exp_and_others). Blank every other set so the greedy
# chooser emits exactly one table load.
_orig_gat = bacc.get_activation_tables
_PREF_SET = "natural_log_exp_and_others"


def _gat_ln_exp_only(arch):
    tables = _orig_gat(arch)
    if _PREF_SET not in tables:
        return tables
    return {
        name: (funcs if name == _PREF_SET else type(funcs)())
        for name, funcs in tables.items()
    }


bacc.get_activation_tables = _gat_ln_exp_only

# per-eval times and RK accumulation weights (w in {1,2})
T_EVALS = [0.0, 0.25, 0.25, 0.5, 0.5, 0.75, 0.75, 1.0]
W_COL = [0, 1, 1, 0, 0, 1, 1, 0]   # caw column: 0 -> (dt/6)c, 1 -> (dt/3)c


def _chunks(width, step=512):
    out = []
    off = 0
    while off < width:
        f = min(step, width - off)
        out.append((off, f))
        off += f
    return out


def build_module(repeat: int = 1):
    nc = bacc.Bacc("TRN2", target_bir_lowering=False, debug=False)
    add = mybir.AluOpType.add
    mult = mybir.AluOpType.mult

    embT = nc.dram_tensor("embT", [E, TOK], _F32R, kind="ExternalInput")
    wxT = nc.dram_tensor("wxT", [E, E], _F32R, kind="ExternalInput")
    mT3 = nc.dram_tensor("mT3", [E, 3 * E], _F32R, kind="ExternalInput")
    hbT = nc.dram_tensor("hbT", [E, SB], _F32, kind="ExternalInput")
    biasV = nc.dram_tensor("biasV", [E, 8], _F32, kind="ExternalInput")
    caw = nc.dram_tensor("caw", [E, 2], _F32, kind="ExternalInput")
    outd = nc.dram_tensor("out", [SB, TOK], _F32, kind="ExternalOutput")

    with tile.TileContext(nc) as tc:
        with (
            tc.tile_pool(name="const", bufs=1) as cp,
            tc.tile_pool(name="stripe", bufs=1) as sp_pool,
            tc.tile_pool(name="ps", bufs=1, space="PSUM") as pp,
            tc.tile_pool(name="ps0", bufs=1, space="PSUM") as pp0,
        ):
            embS = cp.tile([E, TOK], _F32R)
            nc.sync.dma_start(out=embS[:], in_=embT.ap())
            wxS = cp.tile([E, E], _F32R)
            nc.sync.dma_start(out=wxS[:], in_=wxT.ap())
            mS = cp.tile([E, 3 * E], _F32R)
            nc.sync.dma_start(out=mS[:], in_=mT3.ap())
            hbS = cp.tile([E, SB], _F32)
            nc.sync.dma_start(out=hbS[:], in_=hbT.ap())
            bvS = cp.tile([E, 8], _F32)
            nc.sync.dma_start(out=bvS[:], in_=biasV.ap())
            cawS = cp.tile([E, 2], _F32)
            nc.sync.dma_start(out=cawS[:], in_=caw.ap())

            M_HALF = mS[:, 0:E]
            M_DT = mS[:, E:2 * E]
            M_16 = mS[:, 2 * E:3 * E]

            # Touch the gpsimd custom-op library and the activation table
            # BEFORE the repeat loop so their one-time load instructions are
            # emitted outside the loop body (they fire at first use and
            # would otherwise re-execute every iteration).
            warm = cp.tile([E, 2], _F32, name="warm")
            nc.gpsimd.partition_all_reduce(
                warm[:, 0:1], cawS[:, 0:1], channels=E,
                reduce_op=bass_isa.ReduceOp.add)
            nc.scalar.activation(out=warm[:, 1:2], in_=cawS[:, 1:2],
                                 func=AF.Exp, bias=0.0, scale=0.0)

            def emit_iteration():
                # ---- q0 = Wx @ emb (shared by all sb rows), once per repeat
                e0 = cp.tile([E, TOK], _F32R, name="e0")
                ps_init = pp0.tile([E, TOK], _F32, name="ps_init")
                for soff, f in _chunks(TOK):
                    nc.tensor.matmul(ps_init[:, soff:soff + f], wxS[:],
                                     embS[:, soff:soff + f],
                                     start=True, stop=True)
                nc.vector.tensor_copy(out=e0[:], in_=ps_init[:])

                for stripe in range(N_STRIPES):
                    base = sp_pool.tile([E, W_STRIPE], _F32R, name="base", tag="base")
                    S = sp_pool.tile([E, W_STRIPE], _F32R, name="S", tag="S")
                    spT = sp_pool.tile([E, W_STRIPE], _F32R, name="spT", tag="spT")
                    q = sp_pool.tile([E, W_STRIPE], _F32R, name="q", tag="q")
                    sg = sp_pool.tile([E, W_STRIPE], _F32, name="sg", tag="sg")
                    acc = sp_pool.tile([E, W_STRIPE], _F32, name="acc", tag="acc")

                    # base0 per sb row: Identity(e0 + hb_sb)
                    for k in range(4):
                        sb_row = 4 * stripe + k
                        nc.scalar.activation(
                            out=base[:, k * TOK:(k + 1) * TOK], in_=e0[:],
                            func=AF.Identity, bias=hbS[:, sb_row:sb_row + 1],
                            scale=1.0)
                    cur = base  # tile holding q_i for the upcoming eval
                    for step in range(2):
                        for i in range(4):
                            idx = 4 * step + i
                            bias_ap = bvS[:, idx:idx + 1]
                            # sp_i: eval 0 writes S directly (S starts as sp1)
                            if i == 0:
                                sp_dst = S
                            else:
                                sp_dst = spT
                            # e = Exp(q + t*v); sp = Ln(e + 1) = softplus;
                            # s2 = Exp(-sp) = 1 - sigmoid (const folded on
                            # host: out = log_pz0 - s_c + sum_p acc)
                            nc.scalar.activation(
                                out=sg[:], in_=cur[:], func=AF.Exp,
                                bias=bias_ap, scale=1.0)
                            nc.scalar.activation(
                                out=sp_dst[:], in_=sg[:], func=AF.Ln,
                                bias=1.0, scale=1.0)
                            nc.scalar.activation(
                                out=sg[:], in_=sp_dst[:], func=AF.Exp,
                                bias=0.0, scale=-1.0)
                            if idx == 0:
                                # first eval: direct write, no memset needed
                                nc.vector.tensor_scalar(
                                    out=acc[:], in0=sg[:],
                                    scalar1=cawS[:, 0:1], scalar2=None,
                                    op0=mult)
                            else:
                                nc.vector.scalar_tensor_tensor(
                                    out=acc[:], in0=sg[:],
                                    scalar=cawS[:, W_COL[idx]:W_COL[idx] + 1],
                                    in1=acc[:], op0=mult, op1=add)
                            # S combine (evals 1..3)
                            if i == 1 or i == 2:
                                nc.vector.scalar_tensor_tensor(
                                    out=S[:], in0=spT[:], scalar=2.0,
                                    in1=S[:], op0=mult, op1=add)
                            elif i == 3:
                                nc.vector.scalar_tensor_tensor(
                                    out=S[:], in0=spT[:], scalar=1.0,
                                    in1=S[:], op0=mult, op1=add)
                            # matmul pass + combine
                            if i < 3:
                                # q_{i+1} = base + a_i * M @ sp_i
                                w = M_DT if i == 2 else M_HALF
                                src = S if i == 0 else spT
                                dst = q
                            else:
                                # base' = base + (dt/6) * M @ S
                                w = M_16
                                src = S
                                dst = base
                            for moff, mw in _chunks(W_STRIPE, W_MACRO):
                                psq = pp.tile([E, W_MACRO], _F32, name="psq")
                                for soff, f in _chunks(mw):
                                    nc.tensor.matmul(
                                        psq[:, soff:soff + f], w,
                                        src[:, moff + soff:moff + soff + f],
                                        start=True, stop=True)
                                nc.vector.scalar_tensor_tensor(
                                    out=dst[:, moff:moff + mw],
                                    in0=psq[:, :mw], scalar=1.0,
                                    in1=base[:, moff:moff + mw],
                                    op0=mult, op1=add)
                            cur = q if i < 3 else base

                    # divergence: one partition all-reduce, row 0 -> out
                    nc.gpsimd.partition_all_reduce(
                        sg[:], acc[:], channels=E,
                        reduce_op=bass_isa.ReduceOp.add)
                    nc.sync.dma_start(
                        out=outd.ap()[4 * stripe:4 * stripe + 4, :],
                        in_=sg[0:1, :])

            # repeat>1: hardware loop — the body is emitted once and
            # re-executed on device (2x body unrolling measured slightly
            # worse, single body is optimal here).
            if repeat == 1:
                emit_iteration()
            else:
                with tc.For_i(0, repeat):
                    emit_iteration()
    nc.compile()
    return nc


_CACHED_NC = None


def host_prep(h, emb_matrix, log_pz0, Wx, wxt, bx, Wh, wht, bh, W2, b2):
    f = np.float32
    h = np.asarray(h, f)
    emb = np.asarray(emb_matrix, f)
    Wx = np.asarray(Wx, f); wxt = np.asarray(wxt, f); bx = np.asarray(bx, f)
    Wh = np.asarray(Wh, f); wht = np.asarray(wht, f); bh = np.asarray(bh, f)
    W2 = np.asarray(W2, f); b2 = np.asarray(b2, f)

    hb = (h.reshape(SB, E) @ Wh.T + bh + bx).astype(f)           # [16, 128]
    v = (wxt + wht + Wx @ b2).astype(f)                          # [128]
    c = np.einsum("ij,ji->j", W2, Wx).astype(f)                  # [128]
    s_c = f(c.sum(dtype=f))

    M = (Wx @ W2).astype(f)                                      # [128,128]
    mT = np.ascontiguousarray(M.T)
    mT3 = np.concatenate(
        [mT * (DT / 2), mT * DT, mT * (DT / 6)], axis=1).astype(f)

    embT_np = np.ascontiguousarray(emb.T)                        # [128, T]
    wxT_np = np.ascontiguousarray(Wx.T)
    hbT_np = np.ascontiguousarray(hb.T)                          # [128, 16]
    t_arr = np.array(T_EVALS, f)
    biasV_np = np.ascontiguousarray(v[:, None] * t_arr[None, :]).astype(f)
    caw_np = np.ascontiguousarray(
        np.stack([c * (DT / 6), c * (DT / 3)], axis=1)).astype(f)

    in_maps = []
    for core in range(N_CORES):
        in_maps.append({
            "embT": np.ascontiguousarray(
                embT_np[:, core * TOK:(core + 1) * TOK]),
            "wxT": wxT_np,
            "mT3": np.ascontiguousarray(mT3),
            "hbT": hbT_np,
            "biasV": biasV_np,
            "caw": caw_np,
        })
    return in_maps, s_c


def kernel(h, emb_matrix, log_pz0, Wx, wxt, bx, Wh, wht, bh, W2, b2):
    global _CACHED_NC
    if _CACHED_NC is None:
        _CACHED_NC = build_module(repeat=1)
    nc = _CACHED_NC

    in_maps, s_c = host_prep(h, emb_matrix, log_pz0, Wx, wxt, bx,
                             Wh, wht, bh, W2, b2)
    res = run_bass_kernel_spmd(nc, in_maps, list(range(N_CORES)))
    P = np.zeros((SB, T), np.float32)
    for core in range(N_CORES):
        P[:, core * TOK:(core + 1) * TOK] = res.results[core]["out"]
    log_pz0 = np.asarray(log_pz0, np.float32).reshape(SB, T)
    return (log_pz0 - s_c + P).astype(np.float32)
